# revision 1
# baseline (speedup 1.0000x reference)
"""Trainium2 Bass kernel for nn_BondOutputModule (gnn_message_passing).

Reference computation:
    hv = h @ W_out                                   (projection is linear ->
    out[t,b] = sum_{e in type t, graph b} hv[src_e]   pulled before segment sum)
    graph_v[b,t] = out[t,b]; mask; softmax over t

Device strategy (8 cores, SPMD):
  - h row-sharded: each core computes hv for its 12500 nodes (DVE ttr),
    AllGather -> full hv table in DRAM on every core.
  - hv replicated 4x per 16-block into a [6272, 64] f32 table (256B rows) so
    GPSIMD dma_gather (256B-row granularity, int16 row ids) can fetch each
    edge's value; the within-row position is selected with a 16-wide one-hot
    on DVE.
  - Edges of each (type, seg-high) segment are distributed evenly across the
    8 cores so the (t,hi)->chunk grid is identical on every core (SPMD).
  - Per 128-edge chunk: one PE matmul (lhsT = gathered value column [128,1],
    rhs = 16-wide seg-low one-hot) accumulates into PSUM bins
    [base=(t//12)*32, col=(t%12)*256 + hi*16 + lo].
  - AllReduce partial bins [3,3072] (= [36,256] t-major), PE-transpose to
    [256,36], mask + softmax on DVE/ACT.
"""
import sys

if '/opt/trn_rl_repo' not in sys.path:
    sys.path.insert(0, '/opt/trn_rl_repo')

import numpy as np

TRACE = [False]          # test harness can set kernel.TRACE[0] = True
LAST_EXEC_NS = [None]    # filled when TRACE is on

N = 100000
D = 192
T = 36
E = 30000
B = 256
NCORES = 8
NSH = N // NCORES          # 12500 nodes per core
NT = (NSH + 127) // 128    # 98 ttr tiles
NSHP = NT * 128            # 12544 padded nodes per core
NHV = NSHP * NCORES        # 100352 hv table positions
NROW = NHV // 16           # 6272 T4 rows
NBLK = 16                  # gather blocks
Q_PAD = NHV - 1            # position guaranteed to hold 0.0


def _patch_tile_drain():
    """This walrus build accepts at most one sync-wait per CTRL/DMA
    instruction; Tile's tail drain can carry one wait per DMA lane."""
    import concourse.tile as tile
    from concourse.vector_clock import ScopedClock
    from concourse import mybir

    if getattr(tile.TileContext, '_bondout_patched', False):
        return

    def _drain_and_barrier(self, tick_clock, wait_clock):
        nc = self.nc
        carriers = [nc.sync.nop(nofuse=True, hint=f"dw{i}") for i in range(24)]
        drain_inst = nc.sync.drain()
        wait_clock.add_sem_waits(
            drain_inst.ins, ScopedClock({None: tick_clock.global_clock})
        )
        waits = list(drain_inst.ins.sync_info.on_wait)
        if len(waits) > 1:
            drain_inst.ins.sync_info.on_wait = waits[-1:]
            for c, w in zip(carriers, waits[:-1]):
                if c.ins.sync_info is None:
                    c.ins.sync_info = mybir.SyncInfo(on_wait=[w], on_update=[])
                else:
                    c.ins.sync_info.on_wait = [w]
        nc.all_engine_barrier()
        assert self.sems is not None
        popped = nc._tile_sem_poison_stack.pop()
        assert popped is self._sem_poison
        nc.clear_and_free_semaphores(list(self.sems.allocated().values()))
        nc.all_engine_barrier()

    tile.TileContext._drain_and_barrier = _drain_and_barrier
    tile.TileContext._bondout_patched = True


def _split_multi_waits(nc):
    from concourse import mybir
    for f in nc.m.functions:
        for blk in f.blocks:
            new = []
            changed = False
            for inst in blk.instructions:
                si = inst.sync_info
                if si is not None and si.on_wait and len(si.on_wait) > 1:
                    waits = list(si.on_wait)
                    for j, w in enumerate(waits[:-1]):
                        nop = mybir.InstNoOp(
                            name=f"{inst.name}-ws{j}",
                            engine=inst.engine,
                            bass_nofuse=True,
                            sync_info=mybir.SyncInfo(on_wait=[w], on_update=[]),
                        )
                        new.append(nop)
                    si.on_wait = waits[-1:]
                    changed = True
                new.append(inst)
            if changed:
                blk.instructions = new


def _prepare_edges(edge_src, edge_seg):
    """Build the shared chunk grid and per-core slot arrays.

    Returns (chunks, per_core) where chunks is a list of
    (base_partition, psum_col, start, stop, is_pad) shared by all cores, and
    per_core[k] = dict(idx16, off, lo) slot arrays of shape [NCHP, 128].
    """
    src = edge_src.astype(np.int64)
    seg = edge_seg.astype(np.int64)

    # q position of each node in the AllGather'd hv table
    k_n = src // NSH
    nl = src - k_n * NSH
    q = k_n * NSHP + (nl % 128) * NT + (nl // 128)     # [T, E]
    lo_all = seg & 15

    # per (t, hi): segment bounds in the sorted seg rows
    grid = []          # (t, hi, n_chunks)
    seg_bounds = np.empty((T, 17), np.int64)
    for t in range(T):
        seg_bounds[t] = np.searchsorted(seg[t], np.arange(17) * 16)
    counts = seg_bounds[:, 1:] - seg_bounds[:, :-1]            # [T, 16]
    cmax = -(-(counts + NCORES - 1) // NCORES)                 # ceil(n/8)
    nch = np.maximum(1, -(-cmax // 128))                       # chunks per group

    total_real = int(nch.sum())
    nchp = -(-total_real // NBLK) * NBLK                       # pad to NBLK mult
    n_pad_chunks = nchp - total_real

    chunks = []
    qs = np.full((NCORES, nchp, 128), Q_PAD, np.int64)
    lov = np.zeros((NCORES, nchp, 128), np.int64)
    valid = np.zeros((NCORES, nchp, 128), bool)

    ci = 0
    for t in range(T):
        base = (t // 12) * 32
        colbase = (t % 12) * 256
        for hi in range(16):
            a, b = int(seg_bounds[t, hi]), int(seg_bounds[t, hi + 1])
            n = b - a
            g = int(nch[t, hi])
            # split [a, b) across cores as evenly as possible
            splits = np.linspace(a, b, NCORES + 1).round().astype(np.int64)
            for k in range(NCORES):
                ea, eb = int(splits[k]), int(splits[k + 1])
                cn = eb - ea
                if cn > 0:
                    flat = np.arange(cn)
                    cc = ci + flat // 128
                    pp = flat % 128
                    qs[k, cc, pp] = q[t, ea:eb]
                    lov[k, cc, pp] = lo_all[t, ea:eb]
                    valid[k, cc, pp] = True
            for r in range(g):
                chunks.append((base, colbase + hi * 16,
                               r == 0, r == g - 1, False))
            ci += g
    for _ in range(n_pad_chunks):
        chunks.append((0, 0, False, False, True))

    per_core = []
    for k in range(NCORES):
        idx16 = (qs[k] >> 4).astype(np.int16)
        off = np.where(valid[k], qs[k] & 15, 99).astype(np.float32)
        lo = lov[k].astype(np.float32)
        per_core.append({"idx16": idx16, "off": off, "lo": lo})
    return chunks, per_core, nchp


def _wrap_idx(idx16, nchp):
    """dma_gather index layout: per block of PERB idxs, idx i lives at
    partition 16*core + i%16, column i//16, replicated for all 8 Q7 cores."""
    CB = nchp // NBLK
    PERB = CB * 128
    out = np.zeros((128, nchp * 128 // 16), np.int16)
    flat = idx16.reshape(-1)           # slot j = c*128 + p ordering: [c, p]
    for b in range(NBLK):
        blk = flat[b * PERB:(b + 1) * PERB]
        w = blk.reshape(PERB // 16, 16).T      # [16, PERB//16]
        cols = slice(b * (PERB // 16), (b + 1) * (PERB // 16))
        for core in range(8):
            out[core * 16:(core + 1) * 16, cols] = w
    return out


def _build_program(chunks, nchp):
    import concourse.bass as bass
    from concourse import bacc, mybir
    import concourse.tile as tile

    _patch_tile_drain()
    FP = mybir.dt.float32
    I16 = mybir.dt.int16
    CB = nchp // NBLK
    PERB = CB * 128

    nc = bacc.Bacc(num_swdge_queues=4)
    h_in = nc.dram_tensor("h_shard", [NSHP, D], FP, kind="ExternalInput")
    wb_in = nc.dram_tensor("w_bcast", [128, D], FP, kind="ExternalInput")
    idx_in = nc.dram_tensor("idx16", [128, nchp * 128 // 16], I16,
                            kind="ExternalInput")
    off_in = nc.dram_tensor("off", [128, nchp], FP, kind="ExternalInput")
    lo_in = nc.dram_tensor("lo", [128, nchp], FP, kind="ExternalInput")
    iota_in = nc.dram_tensor("iota16", [128, 16], FP, kind="ExternalInput")
    eye_in = nc.dram_tensor("eye36", [36, 36], FP, kind="ExternalInput")
    m0_in = nc.dram_tensor("mask_keep", [128, 72], FP, kind="ExternalInput")
    mn_in = nc.dram_tensor("mask_neg", [128, 72], FP, kind="ExternalInput")
    out_t = nc.dram_tensor("out", [256, 36], FP, kind="ExternalOutput")

    with tile.TileContext(nc) as tc:
        with (tc.tile_pool(name="dram", bufs=1, space="DRAM") as dram,
              tc.tile_pool(name="const", bufs=1) as cp,
              tc.tile_pool(name="hin", bufs=3) as hp,
              tc.tile_pool(name="gath", bufs=4) as gp,
              tc.tile_pool(name="sel", bufs=3) as selp,
              tc.tile_pool(name="psum", bufs=1, space="PSUM") as pp,
              tc.tile_pool(name="fin", bufs=1) as fp_pool):
            # ---------- phase 1: hv ----------
            wt = cp.tile([128, D], FP)
            nc.sync.dma_start(wt[:], wb_in[:])
            iot = cp.tile([128, 16], FP)
            nc.sync.dma_start(iot[:], iota_in[:])
            hvt = cp.tile([128, NT], FP)
            scr = cp.tile([128, D], FP)
            for i in range(NT):
                ht = hp.tile([128, D], FP, tag="h")
                nc.sync.dma_start(ht[:], h_in[i * 128:(i + 1) * 128, :])
                # tensor_tensor_reduce crashes this HW build; use two ops
                nc.vector.tensor_tensor(
                    out=scr[:], in0=ht[:], in1=wt[:],
                    op=mybir.AluOpType.mult)
                nc.vector.tensor_reduce(
                    out=hvt[:, i:i + 1],
                    in_=scr[:].rearrange("p (o d) -> p o d", o=1),
                    axis=mybir.AxisListType.X, op=mybir.AluOpType.add)
            hv_part = dram.tile([NSHP], FP, tag="hvp")
            nc.sync.dma_start(
                hv_part[:].rearrange("(p i) -> p i", p=128), hvt[:])
            hv_full = dram.tile([NHV], FP, tag="hvf")
            nc.gpsimd.collective_compute(
                "AllGather", mybir.AluOpType.bypass,
                replica_groups=[list(range(NCORES))],
                ins=[hv_part.opt()], outs=[hv_full.opt()])

            # ---------- phase 2: T4 table ----------
            hv_sb = cp.tile([128, NHV // 128], FP)      # [128, 784]
            nc.sync.dma_start(
                hv_sb[:], hv_full[:].rearrange("(p x) -> p x", p=128))
            t4_sb = cp.tile([128, (NHV // 128) * 4], FP)  # [128, 3136]
            # broadcast copy: t4_sb[p, i, r, u] = hv_sb[p, 16*i + u]
            nc.vector.tensor_copy(
                out=t4_sb[:].rearrange("p (i r u) -> p i r u", r=4, u=16),
                in_=hv_sb[:].rearrange("p (i o u) -> p i o u", o=1, u=16)
                    .to_broadcast([128, NHV // 2048, 4, 16]))
            t4_dram = dram.tile([NROW, 64], FP, tag="t4")
            nc.sync.dma_start(
                t4_dram[:].rearrange("(p i) u -> p (i u)", p=128), t4_sb[:])

            # ---------- phase 3: gather + select + segsum ----------
            ps = pp.tile([65, 3072], FP)
            iota3 = iot[:].rearrange("p (o c) -> p o c", o=1)
            for b in range(NBLK):
                it = gp.tile([128, PERB // 16], I16, tag="idx")
                nc.sync.dma_start(
                    it[:], idx_in[:, b * (PERB // 16):(b + 1) * (PERB // 16)])
                gt = gp.tile([128, CB, 64], FP, tag="g")
                nc.gpsimd.dma_gather(
                    out_ap=gt[:], in_ap=t4_dram[:], idxs_ap=it[:],
                    num_idxs=PERB, num_idxs_reg=PERB, elem_size=64,
                    single_packet=False, queue_num=b % 4)
                ot = selp.tile([128, CB], FP, tag="off")
                nc.sync.dma_start(ot[:], off_in[:, b * CB:(b + 1) * CB])
                lt = selp.tile([128, CB], FP, tag="lo")
                nc.sync.dma_start(lt[:], lo_in[:, b * CB:(b + 1) * CB])
                oh16 = selp.tile([128, CB * 16], FP, tag="oh16")
                nc.vector.tensor_tensor(
                    out=oh16[:].rearrange("p (c o) -> p c o", o=16),
                    in0=ot[:].to_broadcast([128, CB, 16]),
                    in1=iota3.to_broadcast([128, CB, 16]),
                    op=mybir.AluOpType.is_equal)
                prod = selp.tile([128, CB * 16], FP, tag="prod")
                nc.vector.tensor_tensor(
                    out=prod[:].rearrange("p (c o) -> p c o", o=16),
                    in0=gt[:, :, 0:16],
                    in1=oh16[:].rearrange("p (c o) -> p c o", o=16),
                    op=mybir.AluOpType.mult)
                val = selp.tile([128, CB], FP, tag="val")
                nc.vector.tensor_reduce(
                    out=val[:],
                    in_=prod[:].rearrange("p (c o) -> p c o", o=16),
                    axis=mybir.AxisListType.X, op=mybir.AluOpType.add)
                ohlo = selp.tile([128, CB * 16], FP, tag="ohlo")
                nc.vector.tensor_tensor(
                    out=ohlo[:].rearrange("p (c o) -> p c o", o=16),
                    in0=lt[:].to_broadcast([128, CB, 16]),
                    in1=iota3.to_broadcast([128, CB, 16]),
                    op=mybir.AluOpType.is_equal)
                for j in range(CB):
                    base, col, st, sp_, is_pad = chunks[b * CB + j]
                    nc.tensor.matmul(
                        out=ps[base:base + 1, col:col + 16],
                        lhsT=val[:, j:j + 1],
                        rhs=ohlo[:, j * 16:(j + 1) * 16],
                        start=st, stop=sp_,
                        skip_group_check=is_pad)

            # ---------- phase 4: reduce + softmax ----------
            sb_s = fp_pool.tile([65, 3072], FP, tag="sbs")
            nc.vector.tensor_copy(sb_s[:], ps[:])
            part_d = dram.tile([3, 3072], FP, tag="part")
            nc.sync.dma_start(part_d[:], sb_s[0:65:32, :])
            red_d = dram.tile([3, 3072], FP, tag="red")
            nc.gpsimd.collective_compute(
                "AllReduce", mybir.AluOpType.add,
                replica_groups=[list(range(NCORES))],
                ins=[part_d.opt()], outs=[red_d.opt()])
            a_sb = fp_pool.tile([36, 256], FP, tag="asb")
            nc.sync.dma_start(
                a_sb[:], red_d[:].rearrange("r (tm b) -> (r tm) b", b=256))
            eye = cp.tile([36, 36], FP)
            nc.sync.dma_start(eye[:], eye_in[:])
            m0 = cp.tile([128, 72], FP)
            nc.sync.dma_start(m0[:], m0_in[:])
            mn = cp.tile([128, 72], FP)
            nc.sync.dma_start(mn[:], mn_in[:])
            for g in range(2):
                tp = pp.tile([128, 36], FP, tag="tp")
                nc.tensor.transpose(
                    out=tp[:], in_=a_sb[:, g * 128:(g + 1) * 128],
                    identity=eye[:])
                gv = fp_pool.tile([128, 36], FP, tag="gv")
                nc.vector.tensor_tensor(
                    out=gv[:], in0=tp[:], in1=m0[:, g * 36:(g + 1) * 36],
                    op=mybir.AluOpType.mult)
                nc.vector.tensor_tensor(
                    out=gv[:], in0=gv[:], in1=mn[:, g * 36:(g + 1) * 36],
                    op=mybir.AluOpType.add)
                mx = fp_pool.tile([128, 1], FP, tag="mx")
                nc.vector.tensor_reduce(
                    out=mx[:], in_=gv[:],
                    axis=mybir.AxisListType.X, op=mybir.AluOpType.max)
                gvs = fp_pool.tile([128, 36], FP, tag="gvs")
                nc.vector.tensor_scalar(
                    out=gvs[:], in0=gv[:], scalar1=mx[:], scalar2=None,
                    op0=mybir.AluOpType.subtract)
                ex = fp_pool.tile([128, 36], FP, tag="ex")
                sm = fp_pool.tile([128, 1], FP, tag="sm")
                nc.scalar.activation(
                    out=ex[:], in_=gvs[:],
                    func=mybir.ActivationFunctionType.Exp,
                    accum_out=sm[:])
                rec = fp_pool.tile([128, 1], FP, tag="rec")
                nc.vector.reciprocal(rec[:], sm[:])
                res = fp_pool.tile([128, 36], FP, tag="res")
                nc.vector.tensor_scalar(
                    out=res[:], in0=ex[:], scalar1=rec[:], scalar2=None,
                    op0=mybir.AluOpType.mult)
                nc.sync.dma_start(out_t[g * 128:(g + 1) * 128, :], res[:])

    nc.compile()
    _split_multi_waits(nc)
    return nc


def kernel(h, W_out, edge_src, edge_seg, mask_mat):
    from concourse.bass_utils import run_bass_kernel_spmd

    h = np.ascontiguousarray(h, np.float32)
    W_out = np.ascontiguousarray(W_out, np.float32)
    chunks, per_core, nchp = _prepare_edges(edge_src, edge_seg)

    w_bcast = np.broadcast_to(W_out[:, 0], (128, D)).copy()
    iota16 = np.broadcast_to(np.arange(16, dtype=np.float32), (128, 16)).copy()
    eye36 = np.eye(36, dtype=np.float32)
    def _mask_layout(m):
        return np.ascontiguousarray(
            m.reshape(2, 128, 36).transpose(1, 0, 2).reshape(128, 72))
    mask_keep = _mask_layout((~mask_mat).astype(np.float32))
    mask_neg = _mask_layout(mask_mat.astype(np.float32) * np.float32(-1e9))

    in_maps = []
    for k in range(NCORES):
        hs = np.zeros((NSHP, D), np.float32)
        hs[:NSH] = h[k * NSH:(k + 1) * NSH]
        in_maps.append({
            "h_shard": hs,
            "w_bcast": w_bcast,
            "idx16": _wrap_idx(per_core[k]["idx16"], nchp),
            "off": per_core[k]["off"].T.copy(),   # [128, nchp]
            "lo": per_core[k]["lo"].T.copy(),
            "iota16": iota16,
            "eye36": eye36,
            "mask_keep": mask_keep,
            "mask_neg": mask_neg,
        })

    nc = _build_program(chunks, nchp)
    kwargs = {}
    if TRACE[0]:
        import tempfile
        kwargs = dict(trace=True, tmpdir=tempfile.mkdtemp(prefix="bondout_"))
    res = run_bass_kernel_spmd(nc, in_maps, core_ids=list(range(NCORES)),
                               **kwargs)
    LAST_EXEC_NS[0] = res.exec_time_ns
    return np.asarray(res.results[0]["out"], np.float32)

